# revision 1
# baseline (speedup 1.0000x reference)
"""Multi-head attention (4x2048x512, 8 heads of 64) on 8 Trainium2 NeuronCores.

Sharding: core c handles batch b = c//2 and head-group g = c%2 (4 heads each).
The host pre-transposes x[b] -> xT [512, 2048], slices the QKV / out
projection weights per core, and casts them to bf16.  Each core computes

    qT,kT  = w_qk.T @ xT          (per head, [64, 2048] each)
    v      = xT.T  @ w_v          (natural [2048, 256], +ones column)
    sT     = kT.T  @ qT           (scores transposed, [j, i], fp32 in PSUM)
    p      = exp(sT / 8)          (flash-style over j-chunks, bf16 out)
    oT     = v_aug.T @ p          (accumulated over j; row 64 = softmax denom)
    attT   = oT[0:64] / denom     (denominator kept in fp32)
    yT     = w_out_slice.T @ attT  ([512, 2048] fp32 partial)

and the host reduces: out[b] = (yT[2b] + yT[2b+1]).T + b_out.

All matmuls run in bf16 (fp32 PSUM accumulation).  float32r would be more
accurate but measures ~4-5us per matmul on this hardware (~25x the bf16
rate), so bf16 it is.  The softmax exp runs on the Scalar engine straight
out of PSUM with the 1/8 scale folded into the activation; the softmax
numerator and denominator are both sums of the same bf16-rounded p, so the
normalization cancels most of the rounding error.
"""

import numpy as np

N = 2048          # sequence length
DMODEL = 512      # model dim
DH = 64           # head dim
HEADS = 4         # heads per core
N_CORES = 8
I_HALF = N // 2   # flash loop processes i in halves of 1024
JC = N // 128     # 16 j-chunks per head
KO = DMODEL // 128  # 4 contraction chunks of the model dim

_CACHE = {}


def _fixup_drains(nc, mybir):
    """walrus in this container rejects instructions carrying multiple sem
    waits ("Too many sync wait commands", e.g. on Drain and on the fused
    LDWEIGHTS of Matmult); hoist all-but-one wait onto single-wait NoOps
    right before the instruction — semantically identical (the engine
    stalls at the NoOps instead)."""
    for fn in nc.m.functions:
        for blk in fn.blocks:
            new = []
            for inst in blk.instructions:
                si = getattr(inst, "sync_info", None)
                if si is not None and si.on_wait:
                    keep = 0 if isinstance(inst, mybir.InstDrain) else 1
                    waits = list(si.on_wait)
                    if len(waits) > keep:
                        extra, rest = waits[keep:], waits[:keep]
                        for j, w in enumerate(extra):
                            nop = mybir.InstNoOp(
                                name=f"{inst.name}-ws{j}", ins=[], outs=[]
                            )
                            nop.engine = inst.engine
                            nop.sync_info = mybir.SyncInfo(on_wait=[w], on_update=[])
                            new.append(nop)
                        si.on_wait = rest
                new.append(inst)
            blk.instructions = new


def build_nc(repeat=1, fixup=True, loop=False, stages=4):
    """Build the per-core Bass program (identical on all 8 cores)."""
    import contextlib

    import concourse.bass as bass
    import concourse.tile as tile
    from concourse import mybir

    f32 = mybir.dt.float32
    bf16 = mybir.dt.bfloat16

    nc = bass.Bass()
    xt = nc.dram_tensor("xt", [DMODEL, N], bf16, kind="ExternalInput")
    wqk = nc.dram_tensor("wqk", [DMODEL, HEADS * 128], bf16, kind="ExternalInput")
    wv = nc.dram_tensor("wv", [DMODEL, HEADS * DH], bf16, kind="ExternalInput")
    wo = nc.dram_tensor("wo", [HEADS * DH, DMODEL], bf16, kind="ExternalInput")
    yt = nc.dram_tensor("yt", [DMODEL, N], f32, kind="ExternalOutput")

    with tile.TileContext(nc) as tc:
        with (
            tc.tile_pool(name="singles", bufs=1) as singles,
        ):
            x_sb = singles.tile([128, KO, N], bf16)
            wqk_sb = singles.tile([128, KO, HEADS, 128], bf16)
            wv_sb = singles.tile([128, KO, HEADS * DH], bf16)
            wo_sb = singles.tile([128, 2, DMODEL], bf16)
            q_sb = singles.tile([DH, HEADS, N], bf16)
            k_sb = singles.tile([DH, HEADS, N], bf16)
            v_sb = singles.tile([128, JC, HEADS, 2 * DH], bf16)
            att_sb = singles.tile([128, 2, N], bf16)

            nc.sync.dma_start(x_sb[:], xt.ap().rearrange("(ko p) n -> p ko n", p=128))
            nc.sync.dma_start(
                wqk_sb[:], wqk.ap().rearrange("(ko p) (h m) -> p ko h m", p=128, m=128)
            )
            nc.sync.dma_start(wv_sb[:], wv.ap().rearrange("(ko p) v -> p ko v", p=128))
            nc.sync.dma_start(wo_sb[:], wo.ap().rearrange("(c p) n -> p c n", p=128))
            nc.vector.memset(v_sb[:, :, :, DH + 1 :], 0.0)
            nc.vector.memset(v_sb[:, :, :, DH : DH + 1], 1.0)
            f16 = mybir.dt.float16
            ones_col = singles.tile([1, DH], f16)
            nc.vector.memset(ones_col[:], 1.0)

            loop_cm = tc.For_i(0, repeat, 1) if loop else contextlib.nullcontext()
            with loop_cm:
              for rep in range(1 if loop else repeat):
                # ---- phase A: qkT per head + v (natural layout) ----
                with (
                    tc.tile_pool(name="ps_qk", bufs=1, space="PSUM") as ps_qk,
                    tc.tile_pool(name="ps_v", bufs=4, space="PSUM") as ps_v,
                ):
                    for h in range(HEADS):
                        pqk = ps_qk.tile([128, N], f32)
                        for t in range(N // 512):
                            for ko in range(KO):
                                nc.tensor.matmul(
                                    pqk[:, t * 512 : (t + 1) * 512],
                                    wqk_sb[:, ko, h, :],
                                    x_sb[:, ko, t * 512 : (t + 1) * 512],
                                    start=(ko == 0),
                                    stop=(ko == KO - 1),
                                )
                        nc.vector.tensor_copy(q_sb[:, h, :], pqk[0:DH, :])
                        nc.vector.tensor_copy(k_sb[:, h, :], pqk[DH:128, :])
                    for jc in range(JC):
                        pv = ps_v.tile([128, HEADS * DH], f32)
                        for ko in range(KO):
                            nc.tensor.matmul(
                                pv[:],
                                x_sb[:, ko, jc * 128 : (jc + 1) * 128],
                                wv_sb[:, ko, :],
                                start=(ko == 0),
                                stop=(ko == KO - 1),
                            )
                        nc.vector.tensor_copy(
                            v_sb[:, jc, :, 0:DH],
                            pv[:].rearrange("p (h d) -> p h d", d=DH),
                        )

                # ---- phase B: flash attention over (i_half, head, j-chunk) ----
                if stages < 2:
                    continue
                with (
                    tc.tile_pool(name="ps_s", bufs=2, space="PSUM") as ps_s,
                    tc.tile_pool(name="ps_o", bufs=2, space="PSUM") as ps_o,
                    tc.tile_pool(name="p_sb", bufs=3) as p_pool,
                    tc.tile_pool(name="den_row", bufs=2) as den_pool,
                    tc.tile_pool(name="rep_sb", bufs=2) as rep_pool,
                ):
                    for ih in range(2):
                        i0 = ih * I_HALF
                        for h in range(HEADS):
                            o = ps_o.tile([2 * DH, I_HALF], f32)
                            for jc in range(JC):
                                s = ps_s.tile([128, I_HALF], f32)
                                for t in range(I_HALF // 512):
                                    nc.tensor.matmul(
                                        s[:, t * 512 : (t + 1) * 512],
                                        k_sb[:, h, jc * 128 : (jc + 1) * 128],
                                        q_sb[:, h, i0 + t * 512 : i0 + (t + 1) * 512],
                                        start=True,
                                        stop=True,
                                    )
                                p = p_pool.tile([128, I_HALF], bf16)
                                nc.scalar.activation(
                                    p[:], s[:], mybir.ActivationFunctionType.Exp,
                                    scale=0.125,
                                )
                                for t in range(I_HALF // 512):
                                    nc.tensor.matmul(
                                        o[:, t * 512 : (t + 1) * 512],
                                        v_sb[:, jc, h, :],
                                        p[:, t * 512 : (t + 1) * 512],
                                        start=(jc == 0),
                                        stop=(jc == JC - 1),
                                    )
                            den_row = den_pool.tile([1, I_HALF], f32)
                            nc.vector.tensor_copy(den_row[:], o[DH : DH + 1, :])
                            rec_row = den_pool.tile([1, I_HALF], f16)
                            with nc.allow_low_precision(
                                reason="softmax denom reciprocal; fp16 has "
                                "10-bit mantissa, plenty for a scale factor"
                            ):
                                nc.vector.reciprocal(rec_row[:], den_row[:])
                            # broadcast rec_row across 64 partitions via a
                            # K=1 outer product on the PE (shares the scores
                            # pool's PSUM slots), then one multiply
                            rep_ps = ps_s.tile([DH, I_HALF], f32, tag="s")
                            for t in range(I_HALF // 512):
                                nc.tensor.matmul(
                                    rep_ps[:, t * 512 : (t + 1) * 512],
                                    ones_col[:],
                                    rec_row[:, t * 512 : (t + 1) * 512],
                                    start=True, stop=True,
                                )
                            rep = rep_pool.tile([DH, I_HALF], f32)
                            nc.vector.tensor_copy(rep[:], rep_ps[:])
                            nc.vector.tensor_mul(
                                att_sb[(h % 2) * DH : (h % 2 + 1) * DH, h // 2,
                                       i0 : i0 + I_HALF],
                                o[0:DH, :],
                                rep[:],
                            )

                # ---- phase C: output projection yT = wo.T @ attT ----
                if stages < 4:
                    continue
                with (
                    tc.tile_pool(name="ps_y", bufs=4, space="PSUM") as ps_y,
                    tc.tile_pool(name="y_sb", bufs=2) as y_pool,
                ):
                    for m in range(KO):
                        y_row = y_pool.tile([128, N], f32)
                        for t in range(N // 512):
                            py = ps_y.tile([128, 512], f32)
                            for c in range(2):
                                nc.tensor.matmul(
                                    py[:],
                                    wo_sb[:, c, m * 128 : (m + 1) * 128],
                                    att_sb[:, c, t * 512 : (t + 1) * 512],
                                    start=(c == 0),
                                    stop=(c == 1),
                                )
                            nc.vector.tensor_copy(y_row[:, t * 512 : (t + 1) * 512], py[:])
                        nc.sync.dma_start(
                            yt.ap().rearrange("(mo p) n -> p mo n", p=128)[:, m, :],
                            y_row[:],
                        )

    if fixup:
        _fixup_drains(nc, mybir)
    return nc


def _per_core_inputs(x, w_qkv, w_out):
    """Slice + transpose the full inputs into the 8 per-core input maps."""
    import ml_dtypes

    bf16 = ml_dtypes.bfloat16
    ins = []
    for c in range(N_CORES):
        b, g = c // 2, c % 2
        xt = np.ascontiguousarray(x[b].T).astype(bf16)          # [512, 2048]
        wq = w_qkv[:, g * 256 : (g + 1) * 256]                  # [512, 256]
        wk = w_qkv[:, 512 + g * 256 : 512 + (g + 1) * 256]
        wv = w_qkv[:, 1024 + g * 256 : 1024 + (g + 1) * 256]
        # per head: [w_q_h | w_k_h] -> [512, 4, 128]
        wqk = np.empty((DMODEL, HEADS, 128), np.float32)
        for h in range(HEADS):
            wqk[:, h, :DH] = wq[:, h * DH : (h + 1) * DH]
            wqk[:, h, DH:] = wk[:, h * DH : (h + 1) * DH]
        wo = w_out[g * 256 : (g + 1) * 256, :]                  # [256, 512]
        ins.append(
            {
                "xt": xt,
                "wqk": np.ascontiguousarray(wqk.reshape(DMODEL, HEADS * 128)).astype(bf16),
                "wv": np.ascontiguousarray(wv).astype(bf16),
                "wo": np.ascontiguousarray(wo).astype(bf16),
            }
        )
    return ins


def run_on_hw(x, w_qkv, w_out, b_out, repeat=1, loop=False):
    from concourse.bass_utils import run_bass_kernel_spmd

    key = ("nc", repeat, loop)
    if key not in _CACHE:
        _CACHE[key] = build_nc(repeat=repeat, loop=loop)
    nc = _CACHE[key]
    ins = _per_core_inputs(
        np.asarray(x, np.float32),
        np.asarray(w_qkv, np.float32),
        np.asarray(w_out, np.float32),
    )
    res = run_bass_kernel_spmd(nc, ins, core_ids=list(range(N_CORES)))
    yts = [res.results[c]["yt"] for c in range(N_CORES)]
    b_out = np.asarray(b_out, np.float32)
    out = np.stack(
        [(yts[2 * b] + yts[2 * b + 1]).T + b_out[None, :] for b in range(4)]
    )
    return out.astype(np.float32)


def kernel(x, w_qkv, w_out, b_out):
    return run_on_hw(x, w_qkv, w_out, b_out, repeat=1)



# revision 17
# speedup vs baseline: 330.0580x; 330.0580x over previous
"""Multi-head attention (4x2048x512, 8 heads of 64) on 8 Trainium2 NeuronCores.

Sharding: core c handles batch b = c//2 and head-group g = c%2 (4 heads each).
The host pre-transposes x[b] -> xT [512, 2048], slices the QKV / out
projection weights per core, and casts them to bf16.

Per-core kernel (v2 — paired-head / row-tiled PE layout):

  The core's 4 heads form 2 pairs (2p, 2p+1).  q/k live in SBUF as
  [128, pair, 2048] with the even head's 64 dims on partitions 0-63 and the
  odd head's on 64-127.  The flash loop processes i in chunks of 512 and
  j in chunks of 128:

    s[:, 0:512]    = k[0:64 ].T @ q[0:64 ]   PE tile (0,0)   .. concurrent
    s[:, 512:1024] = k[64:128].T @ q[64:128] PE tile (64,0)  .. (row-tiled)
    p  = exp(s/8)          one [128,1024] Scalar-engine activation (both heads)
    oE += vE_aug.T @ p[:, 0:512]     (vE free layout [v 64 | ones | 0...])
    oO += vO_aug.T @ p[:, 512:1024]  (vO free layout [ones | 0... | v 64])

  so oE rows 0-63 are the even head's numerator with the denominator in row
  64, and oO rows 64-127 are the odd head's numerator with the denominator
  in row 0 — each head's output lands on its own partition range, keeping
  every DVE op partition-aligned.  1/den broadcasts across partitions via a
  K=1 ones-column matmul (two col-tiled mms, one per head).  The out
  projection contracts att [128, pair, 2048] with wo and DMAs fp32 partials;
  the host reduces the two cores per batch and adds b_out.

  The Scalar engine's exp (16.8M elements at ~153G elem/s) is the ~110us
  floor; matmuls are bf16 (fp32 PSUM) with the K=64 score matmuls row-tiled
  so the PE stays under that floor.  Emission order starts the first exps
  ~8us in and hides the qk/v projections and the out projection under the
  exp-bound flash window.
"""

import numpy as np

N = 2048            # sequence length
DMODEL = 512        # model dim
DH = 64             # head dim
HEADS = 4           # heads per core
PAIRS = 2           # head pairs per core
N_CORES = 8
IC = 512            # flash i-chunk
NIC = N // IC       # 4 i-chunks
JC = N // 128       # 16 j-chunks
KO = DMODEL // 128  # 4 contraction chunks of the model dim
T = N // 512        # 4 column chunks for the qk projections

_CACHE = {}

# Timing-probe knob (build variants with wrong numerics but comparable
# instruction streams; never set in production use).
_VARIANT = None


def _fixup_drains(nc, mybir):
    """walrus in this container rejects instructions carrying multiple sem
    waits ("Too many sync wait commands", e.g. on Drain and on the fused
    LDWEIGHTS of Matmult); hoist all-but-one wait onto single-wait NoOps
    right before the instruction — semantically identical (the engine
    stalls at the NoOps instead)."""
    for fn in nc.m.functions:
        for blk in fn.blocks:
            new = []
            for inst in blk.instructions:
                si = getattr(inst, "sync_info", None)
                if si is not None and si.on_wait:
                    keep = 0 if isinstance(inst, mybir.InstDrain) else 1
                    waits = list(si.on_wait)
                    if len(waits) > keep:
                        extra, rest = waits[keep:], waits[:keep]
                        for j, w in enumerate(extra):
                            nop = mybir.InstNoOp(
                                name=f"{inst.name}-ws{j}", ins=[], outs=[]
                            )
                            nop.engine = inst.engine
                            nop.sync_info = mybir.SyncInfo(on_wait=[w], on_update=[])
                            new.append(nop)
                        si.on_wait = rest
                new.append(inst)
            blk.instructions = new


def build_nc(repeat=1, fixup=True, loop=False, unroll=1):
    """Build the per-core Bass program (identical on all 8 cores).

    loop=True wraps the body in a hardware For_i loop of `repeat`
    iterations, with `unroll` body copies per iteration (amortizes the
    ~44us per-back-edge all-engine barrier + sem-reset cost)."""
    import contextlib

    import concourse.bass as bass
    import concourse.tile as tile
    from concourse import mybir

    f32 = mybir.dt.float32
    bf16 = mybir.dt.bfloat16
    f16 = mybir.dt.float16

    nc = bass.Bass()
    xt = nc.dram_tensor("xt", [DMODEL, N], bf16, kind="ExternalInput")
    wqk = nc.dram_tensor("wqk", [DMODEL, PAIRS * 2 * 128], bf16, kind="ExternalInput")
    wv = nc.dram_tensor("wv", [DMODEL, HEADS * DH], bf16, kind="ExternalInput")
    wo = nc.dram_tensor("wo", [HEADS * DH, DMODEL], bf16, kind="ExternalInput")
    yt = nc.dram_tensor("yt", [DMODEL, N], f32, kind="ExternalOutput")

    with tile.TileContext(nc) as tc:
        with tc.tile_pool(name="singles", bufs=1) as singles:
            x_sb = singles.tile([128, KO, N], bf16)
            wqk_sb = singles.tile([128, KO, 4, 128], bf16)
            wv_sb = singles.tile([128, KO, HEADS * DH], bf16)
            wo_sb = singles.tile([128, 2, DMODEL], bf16)
            q_sb = singles.tile([128, PAIRS, N], bf16)
            k_sb = singles.tile([128, PAIRS, N], bf16)
            v_sb = singles.tile([128, JC, HEADS, 128], bf16)
            att_sb = singles.tile([128, PAIRS, N], bf16)
            ones_col = singles.tile([1, DH], f16)

            nc.sync.dma_start(x_sb[:], xt.ap().rearrange("(ko p) n -> p ko n", p=128))
            nc.sync.dma_start(
                wqk_sb[:], wqk.ap().rearrange("(ko p) (g m) -> p ko g m", p=128, m=128)
            )
            nc.sync.dma_start(wv_sb[:], wv.ap().rearrange("(ko p) v -> p ko v", p=128))
            nc.sync.dma_start(wo_sb[:], wo.ap().rearrange("(c p) n -> p c n", p=128))
            nc.vector.memset(ones_col[:], 1.0)
            # even heads: [v 0:64 | ones at 64 | zeros 65:128]
            nc.vector.memset(v_sb[:, :, 0::2, DH : DH + 1], 1.0)
            nc.vector.memset(v_sb[:, :, 0::2, DH + 1 :], 0.0)
            # odd heads: [ones at 0 | zeros 1:64 | v 64:128]
            nc.vector.memset(v_sb[:, :, 1::2, 0:1], 1.0)
            nc.vector.memset(v_sb[:, :, 1::2, 1:DH], 0.0)

            if loop:
                assert repeat % unroll == 0
                loop_cm = tc.For_i(0, repeat // unroll, 1, staggered_reset=True)
            else:
                loop_cm = contextlib.nullcontext()
            with loop_cm:
              for rep in range(unroll if loop else repeat):
                with (
                    tc.tile_pool(name="ps_s", bufs=3, space="PSUM") as spool,
                    tc.tile_pool(name="ps_o", bufs=2, space="PSUM") as opool,
                    tc.tile_pool(name="p_sb", bufs=4) as p_pool,
                    tc.tile_pool(name="rec_sb", bufs=4) as rec_pool,
                    tc.tile_pool(name="rep_sb", bufs=2) as rep_pool,
                    tc.tile_pool(name="y_sb", bufs=2) as y_pool,
                ):
                    def qk_chain(pair, qk, dest):
                        for t in range(T):
                            pq = spool.tile([128, 512], f32, tag="s")
                            for ko in range(KO):
                                nc.tensor.matmul(
                                    pq[:],
                                    wqk_sb[:, ko, pair * 2 + qk, :],
                                    x_sb[:, ko, t * 512 : (t + 1) * 512],
                                    start=(ko == 0),
                                    stop=(ko == KO - 1),
                                )
                            nc.vector.tensor_copy(
                                dest[:, pair, t * 512 : (t + 1) * 512], pq[:]
                            )

                    def v_chain():
                        for jc in range(JC):
                            pv = opool.tile([128, HEADS * DH], f32, tag="o")
                            for ko in range(KO):
                                nc.tensor.matmul(
                                    pv[:],
                                    x_sb[:, ko, jc * 128 : (jc + 1) * 128],
                                    wv_sb[:, ko, :],
                                    start=(ko == 0),
                                    stop=(ko == KO - 1),
                                )
                            pv_r = pv[:].rearrange("p (h2 two d) -> p h2 two d", two=2, d=DH)
                            # even heads -> cols 0:64, odd heads -> cols 64:128
                            nc.vector.tensor_copy(
                                v_sb[:, jc, 0::2, 0:DH], pv_r[:, :, 0, :]
                            )
                            nc.vector.tensor_copy(
                                v_sb[:, jc, 1::2, DH:128], pv_r[:, :, 1, :]
                            )

                    def s_mms(c, pair, jc):
                        """Row-tiled score matmuls for one (i-chunk, pair, jc)."""
                        i0 = c * IC
                        s = spool.tile([128, 2 * IC], f32, tag="s")
                        nc.tensor.matmul(
                            s[:, 0:IC],
                            k_sb[0:64, pair, jc * 128 : (jc + 1) * 128],
                            q_sb[0:64, pair, i0 : i0 + IC],
                            start=True, stop=True,
                            tile_position=(0, 0),
                        )
                        nc.tensor.matmul(
                            s[:, IC : 2 * IC],
                            k_sb[64:128, pair, jc * 128 : (jc + 1) * 128],
                            q_sb[64:128, pair, i0 : i0 + IC],
                            start=True, stop=True,
                            tile_position=(64, 0),
                        )
                        return s

                    def flash(c, pair, hooks=()):
                        """Flash loop, software-pipelined for the in-order PE
                        queue: the score matmuls for jc+2 are emitted BEFORE
                        the o-matmuls for jc, so the PE keeps streaming scores
                        while exp(jc) is in flight on the Scalar engine
                        instead of stalling at o(jc)'s semaphore."""
                        hooks = dict(hooks)
                        i0 = c * IC
                        oE = opool.tile([128, IC], f32, tag="o")
                        oO = opool.tile([128, IC], f32, tag="o")

                        def o_mms(jc, p):
                            nc.tensor.matmul(
                                oE[:],
                                v_sb[:, jc, 2 * pair, :],
                                p[:, 0:IC],
                                start=(jc == 0),
                                stop=(jc == JC - 1),
                            )
                            if _VARIANT != "half_o":
                                nc.tensor.matmul(
                                    oO[:],
                                    v_sb[:, jc, 2 * pair + 1, :],
                                    p[:, IC : 2 * IC],
                                    start=(jc == 0),
                                    stop=(jc == JC - 1),
                                )
                            elif jc == 0:
                                nc.vector.memset(oO[:], 1.0)

                        # o(jc) is emitted one step late (with s(jc+3) already
                        # ahead of it), so by the time the in-order PE queue
                        # reaches o(jc), exp(jc) finished a full step earlier
                        # and the PE never blocks on the cross-engine sem.
                        s_tiles = {0: s_mms(c, pair, 0), 1: s_mms(c, pair, 1)}
                        p_tiles = {}
                        for jc in range(JC):
                            s = s_tiles.pop(jc)
                            p_tiles[jc] = p_pool.tile([128, 2 * IC], bf16, name="p")
                            nc.scalar.activation(
                                p_tiles[jc][:], s[:],
                                mybir.ActivationFunctionType.Exp,
                                scale=0.125,
                            )
                            if jc + 2 < JC:
                                s_tiles[jc + 2] = s_mms(c, pair, jc + 2)
                            if jc - 1 >= 0:
                                o_mms(jc - 1, p_tiles.pop(jc - 1))
                            if jc in hooks:
                                hooks.pop(jc)()
                        o_mms(JC - 1, p_tiles.pop(JC - 1))
                        recE = rec_pool.tile([1, IC], f16)
                        recO = rec_pool.tile([1, IC], f16)
                        with nc.allow_low_precision(
                            reason="softmax denom reciprocal; fp16 has "
                            "10-bit mantissa, plenty for a scale factor"
                        ):
                            nc.vector.reciprocal(recE[:], oE[DH : DH + 1, :])
                            nc.vector.reciprocal(recO[:], oO[0:1, :])
                        rep_ps = spool.tile([128, IC], f32, tag="s")
                        nc.tensor.matmul(
                            rep_ps[0:DH, :], ones_col[:], recE[:],
                            start=True, stop=True,
                        )
                        nc.tensor.matmul(
                            rep_ps[DH:128, :], ones_col[:], recO[:],
                            start=True, stop=True,
                        )
                        rep = rep_pool.tile([128, IC], f32)
                        nc.vector.tensor_copy(rep[:], rep_ps[:])
                        nc.vector.tensor_mul(
                            att_sb[0:DH, pair, i0 : i0 + IC], oE[0:DH, :], rep[0:DH, :]
                        )
                        nc.vector.tensor_mul(
                            att_sb[DH:128, pair, i0 : i0 + IC],
                            oO[DH:128, :],
                            rep[DH:128, :],
                        )

                    def proj(c):
                        i0 = c * IC
                        for m in range(KO):
                            py = spool.tile([128, IC], f32, tag="s")
                            for pr in range(PAIRS):
                                nc.tensor.matmul(
                                    py[:],
                                    wo_sb[:, pr, m * 128 : (m + 1) * 128],
                                    att_sb[:, pr, i0 : i0 + IC],
                                    start=(pr == 0),
                                    stop=(pr == PAIRS - 1),
                                )
                            y = y_pool.tile([128, IC], f32)
                            nc.vector.tensor_copy(y[:], py[:])
                            nc.sync.dma_start(
                                yt.ap().rearrange("(mo p) n -> p mo n", p=128)[
                                    :, m, i0 : i0 + IC
                                ],
                                y[:],
                            )

                    qk_chain(0, 1, k_sb)
                    qk_chain(0, 0, q_sb)
                    v_chain()

                    def emit_pair1():
                        qk_chain(1, 1, k_sb)
                        qk_chain(1, 0, q_sb)

                    # proj(c) is emitted inside flash(c+1, 0) so its matmuls
                    # land in the PE stream after their att dependencies have
                    # had time to resolve (no stall at the proj queue head).
                    flash(0, 0, hooks={5: emit_pair1})
                    flash(0, 1)
                    for c in range(1, NIC):
                        flash(c, 0, hooks={2: (lambda cc=c - 1: proj(cc))})
                        flash(c, 1)
                    proj(NIC - 1)

    if fixup:
        _fixup_drains(nc, mybir)
    return nc


def _per_core_inputs(x, w_qkv, w_out):
    """Slice + transpose the full inputs into the 8 per-core input maps."""
    import ml_dtypes

    bf16 = ml_dtypes.bfloat16
    ins = []
    for c in range(N_CORES):
        b, g = c // 2, c % 2
        xt = np.ascontiguousarray(x[b].T).astype(bf16)          # [512, 2048]
        wq = w_qkv[:, g * 256 : (g + 1) * 256]                  # [512, 256]
        wk = w_qkv[:, 512 + g * 256 : 512 + (g + 1) * 256]
        wv = w_qkv[:, 1024 + g * 256 : 1024 + (g + 1) * 256]
        # [512, pair, {q,k}, 128]; adjacent heads are adjacent 64-col blocks
        wqk = np.empty((DMODEL, PAIRS, 2, 128), np.float32)
        for p in range(PAIRS):
            wqk[:, p, 0, :] = wq[:, p * 128 : (p + 1) * 128]
            wqk[:, p, 1, :] = wk[:, p * 128 : (p + 1) * 128]
        wo = w_out[g * 256 : (g + 1) * 256, :]                  # [256, 512]
        ins.append(
            {
                "xt": xt,
                "wqk": np.ascontiguousarray(
                    wqk.reshape(DMODEL, PAIRS * 2 * 128)
                ).astype(bf16),
                "wv": np.ascontiguousarray(wv).astype(bf16),
                "wo": np.ascontiguousarray(wo).astype(bf16),
            }
        )
    return ins


def run_on_hw(x, w_qkv, w_out, b_out, repeat=1, loop=False):
    from concourse.bass_utils import run_bass_kernel_spmd

    key = ("nc", repeat, loop)
    if key not in _CACHE:
        _CACHE[key] = build_nc(repeat=repeat, loop=loop)
    nc = _CACHE[key]
    ins = _per_core_inputs(
        np.asarray(x, np.float32),
        np.asarray(w_qkv, np.float32),
        np.asarray(w_out, np.float32),
    )
    res = run_bass_kernel_spmd(nc, ins, core_ids=list(range(N_CORES)))
    yts = [res.results[c]["yt"] for c in range(N_CORES)]
    b_out = np.asarray(b_out, np.float32)
    out = np.stack(
        [(yts[2 * b] + yts[2 * b + 1]).T + b_out[None, :] for b in range(4)]
    )
    return out.astype(np.float32)


def kernel(x, w_qkv, w_out, b_out):
    return run_on_hw(x, w_qkv, w_out, b_out, repeat=1)


# revision 21
# speedup vs baseline: 353.6660x; 1.0715x over previous
"""Multi-head attention (4x2048x512, 8 heads of 64) on 8 Trainium2 NeuronCores.

Sharding: core c handles batch b = c//2 and head-group g = c%2 (4 heads each).
The host pre-transposes x[b] -> xT [512, 2048], slices the QKV / out
projection weights per core, and casts them to bf16.

Per-core kernel (v2 — paired-head / row-tiled PE layout):

  The core's 4 heads form 2 pairs (2p, 2p+1).  q/k live in SBUF as
  [128, pair, 2048] with the even head's 64 dims on partitions 0-63 and the
  odd head's on 64-127.  The flash loop processes i in chunks of 512 and
  j in chunks of 128:

    s[:, 0:512]    = k[0:64 ].T @ q[0:64 ]   PE tile (0,0)   .. concurrent
    s[:, 512:1024] = k[64:128].T @ q[64:128] PE tile (64,0)  .. (row-tiled)
    p  = exp(s/8)          one [128,1024] Scalar-engine activation (both heads)
    oE += vE_aug.T @ p[:, 0:512]     (vE free layout [v 64 | ones | 0...])
    oO += vO_aug.T @ p[:, 512:1024]  (vO free layout [ones | 0... | v 64])

  so oE rows 0-63 are the even head's numerator with the denominator in row
  64, and oO rows 64-127 are the odd head's numerator with the denominator
  in row 0 — each head's output lands on its own partition range, keeping
  every DVE op partition-aligned.  1/den broadcasts across partitions via a
  K=1 ones-column matmul (two col-tiled mms, one per head).  The out
  projection contracts att [128, pair, 2048] with wo and DMAs fp32 partials;
  the host reduces the two cores per batch and adds b_out.

  The Scalar engine's exp (16.8M elements at ~153G elem/s) is the ~110us
  floor; matmuls are bf16 (fp32 PSUM) with the K=64 score matmuls row-tiled
  so the PE stays under that floor.  Emission order starts the first exps
  ~8us in and hides the qk/v projections and the out projection under the
  exp-bound flash window.
"""

import numpy as np

N = 2048            # sequence length
DMODEL = 512        # model dim
DH = 64             # head dim
HEADS = 4           # heads per core
PAIRS = 2           # head pairs per core
N_CORES = 8
IC = 512            # flash i-chunk
NIC = N // IC       # 4 i-chunks
JC = N // 128       # 16 j-chunks
KO = DMODEL // 128  # 4 contraction chunks of the model dim
T = N // 512        # 4 column chunks for the qk projections

_CACHE = {}

# Timing-probe knob (build variants with wrong numerics but comparable
# instruction streams; never set in production use).
_VARIANT = None


def _fixup_drains(nc, mybir):
    """walrus in this container rejects instructions carrying multiple sem
    waits ("Too many sync wait commands", e.g. on Drain and on the fused
    LDWEIGHTS of Matmult); hoist all-but-one wait onto single-wait NoOps
    right before the instruction — semantically identical (the engine
    stalls at the NoOps instead)."""
    for fn in nc.m.functions:
        for blk in fn.blocks:
            new = []
            for inst in blk.instructions:
                si = getattr(inst, "sync_info", None)
                if si is not None and si.on_wait:
                    keep = 0 if isinstance(inst, mybir.InstDrain) else 1
                    waits = list(si.on_wait)
                    if len(waits) > keep:
                        extra, rest = waits[keep:], waits[:keep]
                        for j, w in enumerate(extra):
                            nop = mybir.InstNoOp(
                                name=f"{inst.name}-ws{j}", ins=[], outs=[]
                            )
                            nop.engine = inst.engine
                            nop.sync_info = mybir.SyncInfo(on_wait=[w], on_update=[])
                            new.append(nop)
                        si.on_wait = rest
                new.append(inst)
            blk.instructions = new


def build_nc(repeat=1, fixup=True, loop=False, unroll=1):
    """Build the per-core Bass program (identical on all 8 cores).

    loop=True wraps the body in a hardware For_i loop of `repeat`
    iterations, with `unroll` body copies per iteration (amortizes the
    ~44us per-back-edge all-engine barrier + sem-reset cost)."""
    import contextlib

    import concourse.bass as bass
    import concourse.tile as tile
    from concourse import mybir

    f32 = mybir.dt.float32
    bf16 = mybir.dt.bfloat16
    f16 = mybir.dt.float16

    nc = bass.Bass()
    xt = nc.dram_tensor("xt", [DMODEL, N], bf16, kind="ExternalInput")
    wqk = nc.dram_tensor("wqk", [DMODEL, PAIRS * 2 * 128], bf16, kind="ExternalInput")
    wv = nc.dram_tensor("wv", [DMODEL, HEADS * DH], bf16, kind="ExternalInput")
    wo = nc.dram_tensor("wo", [HEADS * DH, DMODEL], bf16, kind="ExternalInput")
    yt = nc.dram_tensor("yt", [DMODEL, N], f32, kind="ExternalOutput")

    with tile.TileContext(nc) as tc:
        with tc.tile_pool(name="singles", bufs=1) as singles:
            x_sb = singles.tile([128, KO, N], bf16)
            wqk_sb = singles.tile([128, KO, 4, 128], bf16)
            wv_sb = singles.tile([128, KO, HEADS * DH], bf16)
            wo_sb = singles.tile([128, 2, DMODEL], bf16)
            q_sb = singles.tile([128, PAIRS, N], bf16)
            k_sb = singles.tile([128, PAIRS, N], bf16)
            v_sb = singles.tile([128, JC, HEADS, 128], bf16)
            att_sb = singles.tile([128, PAIRS, N], bf16)
            ones_col = singles.tile([1, DH], f16)

            nc.sync.dma_start(x_sb[:], xt.ap().rearrange("(ko p) n -> p ko n", p=128))
            nc.sync.dma_start(
                wqk_sb[:], wqk.ap().rearrange("(ko p) (g m) -> p ko g m", p=128, m=128)
            )
            nc.sync.dma_start(wv_sb[:], wv.ap().rearrange("(ko p) v -> p ko v", p=128))
            nc.sync.dma_start(wo_sb[:], wo.ap().rearrange("(c p) n -> p c n", p=128))
            nc.vector.memset(ones_col[:], 1.0)
            # even heads: [v 0:64 | ones at 64 | zeros 65:128]
            nc.vector.memset(v_sb[:, :, 0::2, DH : DH + 1], 1.0)
            nc.vector.memset(v_sb[:, :, 0::2, DH + 1 :], 0.0)
            # odd heads: [ones at 0 | zeros 1:64 | v 64:128]
            nc.vector.memset(v_sb[:, :, 1::2, 0:1], 1.0)
            nc.vector.memset(v_sb[:, :, 1::2, 1:DH], 0.0)

            if loop:
                assert repeat % unroll == 0
                loop_cm = tc.For_i(0, repeat // unroll, 1, staggered_reset=True)
            else:
                loop_cm = contextlib.nullcontext()
            with loop_cm:
              for rep in range(unroll if loop else repeat):
                with (
                    tc.tile_pool(name="ps_s", bufs=3, space="PSUM") as spool,
                    tc.tile_pool(name="ps_o", bufs=2, space="PSUM") as opool,
                    tc.tile_pool(name="p_sb", bufs=4) as p_pool,
                    tc.tile_pool(name="rec_sb", bufs=4) as rec_pool,
                    tc.tile_pool(name="rep_sb", bufs=2) as rep_pool,
                    tc.tile_pool(name="y_sb", bufs=2) as y_pool,
                ):
                    def qk_chain(pair, qk, dest):
                        for t in range(T):
                            pq = spool.tile([128, 512], f32, tag="s")
                            for ko in range(KO):
                                nc.tensor.matmul(
                                    pq[:],
                                    wqk_sb[:, ko, pair * 2 + qk, :],
                                    x_sb[:, ko, t * 512 : (t + 1) * 512],
                                    start=(ko == 0),
                                    stop=(ko == KO - 1),
                                )
                            nc.vector.tensor_copy(
                                dest[:, pair, t * 512 : (t + 1) * 512], pq[:]
                            )

                    def v_chain():
                        for jc in range(JC):
                            pv = opool.tile([128, HEADS * DH], f32, tag="o")
                            for ko in range(KO):
                                nc.tensor.matmul(
                                    pv[:],
                                    x_sb[:, ko, jc * 128 : (jc + 1) * 128],
                                    wv_sb[:, ko, :],
                                    start=(ko == 0),
                                    stop=(ko == KO - 1),
                                )
                            pv_r = pv[:].rearrange("p (h2 two d) -> p h2 two d", two=2, d=DH)
                            # even heads -> cols 0:64, odd heads -> cols 64:128
                            nc.vector.tensor_copy(
                                v_sb[:, jc, 0::2, 0:DH], pv_r[:, :, 0, :]
                            )
                            nc.vector.tensor_copy(
                                v_sb[:, jc, 1::2, DH:128], pv_r[:, :, 1, :]
                            )

                    def s_mms(c, pair, jc):
                        """Row-tiled score matmuls for one (i-chunk, pair, jc)."""
                        i0 = c * IC
                        s = spool.tile([128, 2 * IC], f32, tag="s")
                        nc.tensor.matmul(
                            s[:, 0:IC],
                            k_sb[0:64, pair, jc * 128 : (jc + 1) * 128],
                            q_sb[0:64, pair, i0 : i0 + IC],
                            start=True, stop=True,
                            tile_position=(0, 0),
                        )
                        nc.tensor.matmul(
                            s[:, IC : 2 * IC],
                            k_sb[64:128, pair, jc * 128 : (jc + 1) * 128],
                            q_sb[64:128, pair, i0 : i0 + IC],
                            start=True, stop=True,
                            tile_position=(64, 0),
                        )
                        return s

                    def flash(c, pair, hooks=(), pre_s=None, nxt=None):
                        """Flash loop, software-pipelined for the in-order PE
                        queue: the score matmuls for jc+2 are emitted BEFORE
                        the o-matmuls for jc, so the PE keeps streaming scores
                        while exp(jc) is in flight on the Scalar engine
                        instead of stalling at o(jc)'s semaphore.  The last
                        two steps also emit the NEXT flash's first two score
                        matmuls (returned and passed back in via pre_s), so
                        the Scalar engine sees no bubble at flash boundaries."""
                        hooks = dict(hooks)
                        i0 = c * IC
                        oE = opool.tile([128, IC], f32, tag="o")
                        oO = opool.tile([128, IC], f32, tag="o")

                        def o_mms(jc, p):
                            nc.tensor.matmul(
                                oE[:],
                                v_sb[:, jc, 2 * pair, :],
                                p[:, 0:IC],
                                start=(jc == 0),
                                stop=(jc == JC - 1),
                            )
                            if _VARIANT != "half_o":
                                nc.tensor.matmul(
                                    oO[:],
                                    v_sb[:, jc, 2 * pair + 1, :],
                                    p[:, IC : 2 * IC],
                                    start=(jc == 0),
                                    stop=(jc == JC - 1),
                                )
                            elif jc == 0:
                                nc.vector.memset(oO[:], 1.0)

                        # o(jc) is emitted one step late (with s(jc+3) already
                        # ahead of it), so by the time the in-order PE queue
                        # reaches o(jc), exp(jc) finished a full step earlier
                        # and the PE never blocks on the cross-engine sem.
                        if pre_s is None:
                            s_tiles = {0: s_mms(c, pair, 0), 1: s_mms(c, pair, 1)}
                        else:
                            s_tiles = pre_s
                        next_s = {}
                        p_tiles = {}
                        for jc in range(JC):
                            s = s_tiles.pop(jc)
                            p_tiles[jc] = p_pool.tile([128, 2 * IC], bf16, name="p")
                            nc.scalar.activation(
                                p_tiles[jc][:], s[:],
                                mybir.ActivationFunctionType.Exp,
                                scale=0.125,
                            )
                            if jc + 2 < JC:
                                s_tiles[jc + 2] = s_mms(c, pair, jc + 2)
                            elif nxt is not None:
                                next_s[jc + 2 - JC] = s_mms(nxt[0], nxt[1], jc + 2 - JC)
                            if jc - 1 >= 0:
                                o_mms(jc - 1, p_tiles.pop(jc - 1))
                            if jc in hooks:
                                hooks.pop(jc)()
                        o_mms(JC - 1, p_tiles.pop(JC - 1))
                        recE = rec_pool.tile([1, IC], f16)
                        recO = rec_pool.tile([1, IC], f16)
                        with nc.allow_low_precision(
                            reason="softmax denom reciprocal; fp16 has "
                            "10-bit mantissa, plenty for a scale factor"
                        ):
                            nc.vector.reciprocal(recE[:], oE[DH : DH + 1, :])
                            nc.vector.reciprocal(recO[:], oO[0:1, :])
                        rep_ps = spool.tile([128, IC], f32, tag="s")
                        nc.tensor.matmul(
                            rep_ps[0:DH, :], ones_col[:], recE[:],
                            start=True, stop=True,
                        )
                        nc.tensor.matmul(
                            rep_ps[DH:128, :], ones_col[:], recO[:],
                            start=True, stop=True,
                        )
                        rep = rep_pool.tile([128, IC], f32)
                        nc.vector.tensor_copy(rep[:], rep_ps[:])
                        nc.vector.tensor_mul(
                            att_sb[0:DH, pair, i0 : i0 + IC], oE[0:DH, :], rep[0:DH, :]
                        )
                        nc.vector.tensor_mul(
                            att_sb[DH:128, pair, i0 : i0 + IC],
                            oO[DH:128, :],
                            rep[DH:128, :],
                        )
                        return next_s

                    def proj(c):
                        i0 = c * IC
                        for m in range(KO):
                            py = spool.tile([128, IC], f32, tag="s")
                            for pr in range(PAIRS):
                                nc.tensor.matmul(
                                    py[:],
                                    wo_sb[:, pr, m * 128 : (m + 1) * 128],
                                    att_sb[:, pr, i0 : i0 + IC],
                                    start=(pr == 0),
                                    stop=(pr == PAIRS - 1),
                                )
                            y = y_pool.tile([128, IC], f32)
                            nc.vector.tensor_copy(y[:], py[:])
                            nc.sync.dma_start(
                                yt.ap().rearrange("(mo p) n -> p mo n", p=128)[
                                    :, m, i0 : i0 + IC
                                ],
                                y[:],
                            )

                    qk_chain(0, 1, k_sb)
                    qk_chain(0, 0, q_sb)
                    v_chain()

                    def emit_pair1():
                        qk_chain(1, 1, k_sb)
                        qk_chain(1, 0, q_sb)

                    # proj(c) is emitted inside flash(c+1, 0) so its matmuls
                    # land in the PE stream after their att dependencies have
                    # had time to resolve (no stall at the proj queue head).
                    # Each flash also prefetches the next flash's first two
                    # score matmuls (pre_s/nxt) to kill boundary bubbles.
                    seq = [(c, pr) for c in range(NIC) for pr in range(PAIRS)]
                    all_hooks = {0: {5: emit_pair1}}
                    for i in range(2, len(seq), 2):
                        c = seq[i][0]
                        all_hooks[i] = {2: (lambda cc=c - 1: proj(cc))}
                    pre = None
                    for i, (c, pr) in enumerate(seq):
                        nxt = seq[i + 1] if i + 1 < len(seq) else None
                        pre = flash(
                            c, pr, hooks=all_hooks.get(i, ()), pre_s=pre, nxt=nxt
                        )
                    proj(NIC - 1)

    if fixup:
        _fixup_drains(nc, mybir)
    return nc


def _per_core_inputs(x, w_qkv, w_out):
    """Slice + transpose the full inputs into the 8 per-core input maps."""
    import ml_dtypes

    bf16 = ml_dtypes.bfloat16
    ins = []
    for c in range(N_CORES):
        b, g = c // 2, c % 2
        xt = np.ascontiguousarray(x[b].T).astype(bf16)          # [512, 2048]
        wq = w_qkv[:, g * 256 : (g + 1) * 256]                  # [512, 256]
        wk = w_qkv[:, 512 + g * 256 : 512 + (g + 1) * 256]
        wv = w_qkv[:, 1024 + g * 256 : 1024 + (g + 1) * 256]
        # [512, pair, {q,k}, 128]; adjacent heads are adjacent 64-col blocks
        wqk = np.empty((DMODEL, PAIRS, 2, 128), np.float32)
        for p in range(PAIRS):
            wqk[:, p, 0, :] = wq[:, p * 128 : (p + 1) * 128]
            wqk[:, p, 1, :] = wk[:, p * 128 : (p + 1) * 128]
        wo = w_out[g * 256 : (g + 1) * 256, :]                  # [256, 512]
        ins.append(
            {
                "xt": xt,
                "wqk": np.ascontiguousarray(
                    wqk.reshape(DMODEL, PAIRS * 2 * 128)
                ).astype(bf16),
                "wv": np.ascontiguousarray(wv).astype(bf16),
                "wo": np.ascontiguousarray(wo).astype(bf16),
            }
        )
    return ins


def run_on_hw(x, w_qkv, w_out, b_out, repeat=1, loop=False):
    from concourse.bass_utils import run_bass_kernel_spmd

    key = ("nc", repeat, loop)
    if key not in _CACHE:
        _CACHE[key] = build_nc(repeat=repeat, loop=loop)
    nc = _CACHE[key]
    ins = _per_core_inputs(
        np.asarray(x, np.float32),
        np.asarray(w_qkv, np.float32),
        np.asarray(w_out, np.float32),
    )
    res = run_bass_kernel_spmd(nc, ins, core_ids=list(range(N_CORES)))
    yts = [res.results[c]["yt"] for c in range(N_CORES)]
    b_out = np.asarray(b_out, np.float32)
    out = np.stack(
        [(yts[2 * b] + yts[2 * b + 1]).T + b_out[None, :] for b in range(4)]
    )
    return out.astype(np.float32)


def kernel(x, w_qkv, w_out, b_out):
    return run_on_hw(x, w_qkv, w_out, b_out, repeat=1)
